# revision 49
# baseline (speedup 1.0000x reference)
"""RegionLoss (YOLOv2) Trainium2 kernel — 8-core batch-parallel SPMD, v2.

Contract: kernel(**inputs) takes FULL inputs (output [32,425,76,76] f32,
target [32,250] f32) and returns the FULL scalar loss, matching
reference.region_loss. Batch is sharded 4 images/core across 8 NeuronCores;
each core computes its partial loss on device; host sums the 8 partials.

Layout/engine strategy (derived from perfetto traces across 9 variants):
 - GpSimd(Pool) shares SBUF read/write ports with DVE: any concurrent Pool
   op drops DVE throughput up to 5x. Pool only issues DMAs (no ALU work).
 - DVE TensorScalar ops reach ~0.3ns/elem on packed fp16 (4x mode, incl.
   per-partition-scalar fetch); TensorTensor is capped at 2x_1p
   (~0.5ns/elem, ~0.42 at 2F width); ACT costs ~1.1ns/elem regardless.
   Per-op costs inflate ~16% when both DVE and ACT run >90% busy, so the
   engine split matters more than naive balance.
 - The 50-GT loop processes GTs in QUADS (48 GTs; 2 tail GTs at F): the
   per-GT tensor_scalar/ACT ops (which need per-partition scalars) write
   quarters of [128, 4F] tiles; the scalar-free tensor_tensor ops (ox, oy,
   prod, acc-max) and the relu run once per quad at 4F width.
   Per GT t (x side; y symmetric, no relu):
     t0  = (pxr min Gxr_t) - Gxl_t     (DVE TS — both scalar slots fused)
     t1  = relu(pxl - Gxl_t)           (ACT Relu, bias = -Gxl_t)
     ox  = t0 - t1                     (= min(pxr,Gxr) - max(pxl,Gxl))
     oxr = relu(ox)                    (ACT, quad 4F)
     prod= oxr * oy                    (DVE TT, quad 4F)
     u   = prod - 0.375*garea_t        (half ACT Identity+bias, half DVE)
     acc = max(acc, u)                 (DVE TT, quad 4F accumulator)
   suppression test: acc > 1.5*phw*phh  <=>  max-IoU > 0.6 (division-free,
   exact: iou>0.6 <=> inter > 0.375*(parea+garea)).
 - All 128 partitions active: partitions are image-major (32/image), each
   partition holds F=904 consecutive cells of the image's (anchor, hw)
   flattened plane; anchor constants enter as per-cell fp16 tiles.
 - DMAs are split per-channel and queued loop-critical-first (x, y, grid).
 - Activation tables: the 3 Sigmoids run back-to-back first; everything
   else (Exp, Ln, Relu, Identity, Square) shares natural_log_exp's table.
 - The matched-cell "small stage" is issued before the loop so the
   scheduler drops its ops into early engine bubbles.

Everything derived from `target` (gt boxes, best anchors, scatter cells,
masks) is host-precomputed metadata passed as small input tensors; the
<=50 matched cells per image are host-gathered (layout only) and their
coord/conf/class terms computed on device in f32 (small stage).
"""

import math
import numpy as np

# ---- problem constants (hardcoded per contract) ----
NB, NH, NW = 32, 76, 76
NA, NCLS = 5, 80
MAXT = 50
ANCHORS = np.array([1.3221, 1.73145, 3.19275, 4.00944, 5.05587, 8.09892,
                    9.47112, 4.84053, 11.2364, 10.0071], dtype=np.float32)
AW = ANCHORS.reshape(NA, 2)[:, 0]
AH = ANCHORS.reshape(NA, 2)[:, 1]
COORD_SCALE, NOOBJ_SCALE, OBJ_SCALE, CLASS_SCALE = 1.0, 1.0, 5.0, 1.0
THRESH = 0.6

NCORES = 8
BPC = NB // NCORES          # 4 images per core
HW = NH * NW                # 5776
CPI = NA * HW               # 28880 cells per image
PPI = 32                    # partitions per image
F = 904                     # cells per partition; 32*904 = 28928 >= 28880
NPART = 128
NCELL_CAP = 256             # small-stage cell capacity (2 x 128)

_PROG_CACHE = {}


def _build_program():
    import concourse.bacc as bacc
    import concourse.mybir as mybir
    from concourse.tile import TileContext

    f32 = mybir.dt.float32
    f16 = mybir.dt.float16
    Alu = mybir.AluOpType
    Act = mybir.ActivationFunctionType
    X = mybir.AxisListType.X

    nc = bacc.Bacc()

    # ---- I/O ----
    # chans columns: [x | y | conf | w | h] * F
    # consts columns: [gcol | grow | aw/2 | ah/2 | mask] * F
    chans = nc.declare_dram_parameter("chans", [NPART, 5 * F], f32, isOutput=False)
    consts = nc.declare_dram_parameter("consts", [NPART, 5 * F], f16, isOutput=False)
    gtt_d = nc.declare_dram_parameter("gtt", [NPART, 384], f32, isOutput=False)
    gath = nc.declare_dram_parameter("gath", [NCELL_CAP, 85], f32, isOutput=False)
    auxc = nc.declare_dram_parameter("auxc", [NCELL_CAP, 16], f32, isOutput=False)
    oneh = nc.declare_dram_parameter("oneh", [NCELL_CAP, NCLS], f32, isOutput=False)
    out_d = nc.declare_dram_parameter("out", [1, 16], f32, isOutput=True)

    with TileContext(nc) as tc:
        with tc.tile_pool(name="per", bufs=1) as per, \
             tc.tile_pool(name="tmp", bufs=4) as tmp, \
             tc.tile_pool(name="ps", bufs=1, space="PSUM") as ps:

            # ---------- input DMAs (split across queues for overlap) ----------
            cht = per.tile([NPART, 5 * F], f32)
            cst = per.tile([NPART, 5 * F], f16)
            gtt = per.tile([NPART, 384], f32)

            def chd(k):
                nc.gpsimd.dma_start(out=cht[:, k * F:(k + 1) * F],
                                    in_=chans[:, k * F:(k + 1) * F])

            def csd(k):
                nc.gpsimd.dma_start(out=cst[:, k * F:(k + 1) * F],
                                    in_=consts[:, k * F:(k + 1) * F])

            # priority order: loop-critical tensors first
            chd(0); chd(1); csd(0); csd(1)          # x, y, gcol, grow
            chd(3); chd(4); csd(2); csd(3)          # w, h, aw/2, ah/2
            chd(2)                                  # conf
            nc.gpsimd.dma_start(out=gtt[:], in_=gtt_d[:, :])
            csd(4)                                  # mask (end-only)
            g_ts, a_ts, o_ts = [], [], []
            for half in range(2):
                rows = slice(half * 128, (half + 1) * 128)
                g_t = per.tile([128, 85], f32, name=f"g_{half}")
                nc.gpsimd.dma_start(out=g_t[:], in_=gath[rows, :])
                a_t = per.tile([128, 16], f32, name=f"a_{half}")
                nc.gpsimd.dma_start(out=a_t[:], in_=auxc[rows, :])
                o_t = per.tile([128, NCLS], f32, name=f"o_{half}")
                nc.gpsimd.dma_start(out=o_t[:], in_=oneh[rows, :])
                g_ts.append(g_t); a_ts.append(a_t); o_ts.append(o_t)

            # ---------- hoist: sigmoid via 1/(1+exp(-v)) so the WHOLE
            # program lives in the natural_log_exp activation table (one
            # ACT_TABLE_LOAD; no sigmoid-table switch on the critical path).
            # The ts/reciprocal steps run in the ramp where DVE is idle.
            sxyc = per.tile([NPART, 3 * F], f16)   # sigmoid(x|y|conf)
            for k in range(3):
                sl = slice(k * F, (k + 1) * F)
                nc.scalar.activation(sxyc[:, sl], cht[:, sl], Act.Exp,
                                     scale=-1.0)
                nc.vector.tensor_scalar(sxyc[:, sl], sxyc[:, sl], 1.0,
                                        None, Alu.add)
                with nc.allow_low_precision(
                        reason="sigmoid in fp16; loss tolerance is 2e-2"):
                    nc.vector.reciprocal(sxyc[:, sl], sxyc[:, sl])
            ewh = per.tile([NPART, 2 * F], f16)    # exp(w|h)
            nc.scalar.activation(ewh[:], cht[:, 3 * F:5 * F], Act.Exp)

            # wide DVE assembly: [x|y]-paired tiles
            PW = per.tile([NPART, 2 * F], f16)     # [pw/2 | ph/2]
            nc.vector.tensor_tensor(PW[:], ewh[:], cst[:, 2 * F:4 * F],
                                    Alu.mult)
            PC = per.tile([NPART, 2 * F], f16)     # [pxc | pyc]
            nc.vector.tensor_tensor(PC[:], sxyc[:, 0:2 * F], cst[:, 0:2 * F],
                                    Alu.add)
            PR = per.tile([NPART, 2 * F], f16)     # [pxr | pyr]
            nc.vector.tensor_tensor(PR[:], PC[:], PW[:], Alu.add)
            PL = per.tile([NPART, 2 * F], f16)     # [pxl | pyl]
            nc.vector.tensor_tensor(PL[:], PC[:], PW[:], Alu.subtract)
            pxr, pyr = PR[:, 0:F], PR[:, F:2 * F]
            pxl, pyl = PL[:, 0:F], PL[:, F:2 * F]
            # cf2m = (sigmoid(conf)*mask)^2 = conf^2 * mask  (mask is 0/1)
            smk = per.tile([NPART, F], f16)
            nc.vector.tensor_tensor(smk[:], sxyc[:, 2 * F:3 * F],
                                    cst[:, 4 * F:5 * F], Alu.mult)
            cf2m = per.tile([NPART, F], f16)
            nc.scalar.activation(cf2m[:], smk[:], Act.Square)
            pp = per.tile([NPART, F], f16)
            nc.vector.tensor_tensor(pp[:], PW[:, 0:F], PW[:, F:2 * F],
                                    Alu.mult)
            thr = per.tile([NPART, F], f16)   # 1.5*phw*phh
            nc.vector.tensor_scalar(thr[:], pp[:], 1.5, None, Alu.mult)

            scrap = sxyc[:, 0:F]  # sxyc is dead after PC/smk; reuse
            rhs8 = per.tile([NPART, 8], f32)  # all partial sums, one matmul
            nc.gpsimd.memset(rhs8[:], 0.0)
            ones = per.tile([NPART, 1], f32)
            nc.gpsimd.memset(ones[:], 1.0)

            # ---------- small stage: matched cells (DVE + ACT, f32) ----------
            for half in range(2):
                g_t, a_t, o_t = g_ts[half], a_ts[half], o_ts[half]
                # gath cols: 0 x | 1 y | 2 conf | 3 w | 4 h | 5:85 cls
                # auxc cols: 0 gi | 1 gj | 2 lnawh | 3 lnahh | 4 gxl | 5 gxr
                #            6 gyl | 7 gyr | 8 garea | 9 tx | 10 ty | 11 tw
                #            12 th | 13 valid
                # sigmoid via exp table: sig = 1/(1+exp(-v))
                nexp3 = per.tile([128, 3], f32, name=f"nexp3_{half}")
                nc.scalar.activation(nexp3[:], g_t[:, 0:3], Act.Exp,
                                     scale=-1.0)
                d3 = per.tile([128, 3], f32, name=f"d3_{half}")
                nc.vector.tensor_scalar(d3[:], nexp3[:], 1.0, None, Alu.add)
                sig3 = per.tile([128, 3], f32, name=f"sig3_{half}")
                nc.vector.reciprocal(sig3[:], d3[:])
                sphw = per.tile([128, 1], f32, name=f"sphw_{half}")
                nc.scalar.activation(sphw[:], g_t[:, 3:4], Act.Exp,
                                     bias=a_t[:, 2:3])
                sphh = per.tile([128, 1], f32, name=f"sphh_{half}")
                nc.scalar.activation(sphh[:], g_t[:, 4:5], Act.Exp,
                                     bias=a_t[:, 3:4])
                px = per.tile([128, 1], f32, name=f"px_{half}")
                nc.vector.tensor_scalar(px[:], sig3[:, 0:1], a_t[:, 0:1],
                                        None, Alu.add)
                py = per.tile([128, 1], f32, name=f"py_{half}")
                nc.vector.tensor_scalar(py[:], sig3[:, 1:2], a_t[:, 1:2],
                                        None, Alu.add)
                # overlap x
                spxr = per.tile([128, 1], f32, name=f"spxr_{half}")
                nc.vector.tensor_tensor(spxr, px[:], sphw[:], Alu.add)
                spxl = per.tile([128, 1], f32, name=f"spxl_{half}")
                nc.vector.tensor_tensor(spxl, px[:], sphw[:], Alu.subtract)
                st0 = per.tile([128, 1], f32, name=f"st0_{half}")
                nc.vector.tensor_scalar(st0[:], spxr, a_t[:, 5:6],
                                        None, Alu.min)
                st1 = per.tile([128, 1], f32, name=f"st1_{half}")
                nc.vector.tensor_scalar(st1[:], spxl, a_t[:, 4:5],
                                        None, Alu.max)
                sox = per.tile([128, 1], f32, name=f"sox_{half}")
                nc.vector.tensor_tensor(sox[:], st0[:], st1[:], Alu.subtract)
                soxr = per.tile([128, 1], f32, name=f"soxr_{half}")
                nc.vector.tensor_scalar(soxr[:], sox[:], 0.0, None, Alu.max)
                # overlap y
                pyr2 = per.tile([128, 1], f32, name=f"pyr2_{half}")
                nc.vector.tensor_tensor(pyr2[:], py[:], sphh[:], Alu.add)
                pyl2 = per.tile([128, 1], f32, name=f"pyl2_{half}")
                nc.vector.tensor_tensor(pyl2[:], py[:], sphh[:], Alu.subtract)
                st2 = per.tile([128, 1], f32, name=f"st2_{half}")
                nc.vector.tensor_scalar(st2[:], pyr2[:], a_t[:, 7:8],
                                        None, Alu.min)
                st3 = per.tile([128, 1], f32, name=f"st3_{half}")
                nc.vector.tensor_scalar(st3[:], pyl2[:], a_t[:, 6:7],
                                        None, Alu.max)
                soy = per.tile([128, 1], f32, name=f"soy_{half}")
                nc.vector.tensor_tensor(soy[:], st2[:], st3[:], Alu.subtract)
                soyr = per.tile([128, 1], f32, name=f"soyr_{half}")
                nc.vector.tensor_scalar(soyr[:], soy[:], 0.0, None, Alu.max)

                inter = per.tile([128, 1], f32, name=f"inter_{half}")
                nc.vector.tensor_tensor(inter[:], soxr[:], soyr[:], Alu.mult)
                pa = per.tile([128, 1], f32, name=f"pa_{half}")
                nc.vector.tensor_tensor(pa[:], sphw[:], sphh[:], Alu.mult)
                un = per.tile([128, 1], f32, name=f"un_{half}")
                nc.vector.tensor_scalar(un[:], pa[:], 4.0, a_t[:, 8:9],
                                        Alu.mult, Alu.add)
                un2 = per.tile([128, 1], f32, name=f"un2_{half}")
                nc.vector.tensor_tensor(un2[:], un[:], inter[:], Alu.subtract)
                rec = per.tile([128, 1], f32, name=f"rec_{half}")
                nc.vector.reciprocal(rec[:], un2[:])
                tiou = per.tile([128, 1], f32, name=f"tiou_{half}")
                nc.vector.tensor_tensor(tiou[:], inter[:], rec[:], Alu.mult)

                ctb = per.tile([128, 3], f32, name=f"ctb_{half}")
                # coord: 0.5*((sx-tx)^2+(sy-ty)^2+(w-tw)^2+(h-th)^2)
                scr = per.tile([128, 4], f32, name=f"scr_{half}")
                nc.vector.tensor_tensor(scr[:, 0:2], sig3[:, 0:2],
                                        a_t[:, 9:11], Alu.subtract)
                nc.vector.tensor_tensor(scr[:, 2:4], g_t[:, 3:5],
                                        a_t[:, 11:13], Alu.subtract)
                sq4 = per.tile([128, 4], f32, name=f"sq4_{half}")
                nc.scalar.activation(sq4[:], scr[:], Act.Square,
                                     scale=math.sqrt(0.5 * COORD_SCALE))
                nc.vector.tensor_reduce(ctb[:, 0:1], sq4[:], X, Alu.add)
                # obj conf: 2.5*(sconf - tiou)^2
                dcf = per.tile([128, 1], f32, name=f"dcf_{half}")
                nc.vector.tensor_tensor(dcf[:], sig3[:, 2:3], tiou[:],
                                        Alu.subtract)
                nc.scalar.activation(ctb[:, 1:2], dcf[:], Act.Square,
                                     scale=math.sqrt(0.5 * OBJ_SCALE))
                # class CE: logsumexp(cls) - <cls, onehot>
                mx = per.tile([128, 1], f32, name=f"mx_{half}")
                nc.vector.tensor_reduce(mx[:], g_t[:, 5:85], X, Alu.max)
                nmx = per.tile([128, 1], f32, name=f"nmx_{half}")
                nc.vector.tensor_scalar(nmx[:], mx[:], -1.0, None, Alu.mult)
                esc = per.tile([128, NCLS], f32, name=f"esc_{half}")
                sume = per.tile([128, 1], f32, name=f"sume_{half}")
                nc.scalar.activation(esc[:], g_t[:, 5:85], Act.Exp,
                                     bias=nmx[:])
                nc.vector.tensor_reduce(sume[:], esc[:], X, Alu.add)
                lns = per.tile([128, 1], f32, name=f"lns_{half}")
                nc.scalar.activation(lns[:], sume[:], Act.Ln)
                lse = per.tile([128, 1], f32, name=f"lse_{half}")
                nc.vector.tensor_tensor(lse[:], lns[:], mx[:], Alu.add)
                tgl = per.tile([128, NCLS], f32, name=f"tgl_{half}")
                tgv = per.tile([128, 1], f32, name=f"tgv_{half}")
                nc.vector.tensor_tensor(tgl[:], g_t[:, 5:85], o_t[:], Alu.mult)
                nc.vector.tensor_reduce(tgv[:], tgl[:], X, Alu.add)
                nc.vector.tensor_tensor(ctb[:, 2:3], lse[:], tgv[:],
                                        Alu.subtract)
                nc.vector.tensor_scalar(rhs8[:, 1 + 3 * half:4 + 3 * half],
                                        ctb[:], a_t[:, 13:14], None, Alu.mult)


            acc = smk  # smk is dead after cf2m; reuse its [F] slot

            # ---------- 50-gt inner loop (DVE + ACT only) ----------
            # gtt columns: [0:50) gxr | [50:100) gxl | [100:150) gyr
            #   [150:200) gyl | [200:250) -gxl | [250:300) -gyl
            #   [300:350) -0.375*gw*gh
            # GTs processed in QUADS: per-GT tensor_scalar/ACT ops (which
            # need per-partition scalars) write quarters of [128, 4F]
            # tiles; the scalar-free tensor_tensor ops (ox, oy, prod,
            # acc-max) and the relu run once per quad at 4F width.
            ACC4 = per.tile([NPART, 4 * F], f16)
            ACC4b = per.tile([NPART, 4 * F], f16)
            accs = [ACC4, ACC4b]
            for po in range(MAXT // 4):
                ts4 = (4 * po, 4 * po + 1, 4 * po + 2, 4 * po + 3)
                T02 = tmp.tile([NPART, 4 * F], f16, tag="T02", bufs=2)
                T13 = tmp.tile([NPART, 4 * F], f16, tag="T13", bufs=2)
                T2y = tmp.tile([NPART, 4 * F], f16, tag="T2y", bufs=2)
                T3y = tmp.tile([NPART, 4 * F], f16, tag="T3y", bufs=2)
                for hi, t in enumerate(ts4):
                    sl = slice(hi * F, (hi + 1) * F)
                    nc.vector.tensor_scalar(T02[:, sl], pxr, gtt[:, t:t + 1],
                                            gtt[:, 50 + t:51 + t],
                                            Alu.min, Alu.subtract)
                    nc.scalar.activation(T13[:, sl], pxl, Act.Relu,
                                         bias=gtt[:, 200 + t:201 + t])
                    if t % 7 < 4:
                        nc.vector.tensor_scalar(T2y[:, sl], pyr,
                                                gtt[:, 100 + t:101 + t],
                                                gtt[:, 150 + t:151 + t],
                                                Alu.min, Alu.subtract)
                        nc.scalar.activation(T3y[:, sl], pyl, Act.Relu,
                                             bias=gtt[:, 250 + t:251 + t])
                    else:
                        nc.vector.tensor_scalar(T2y[:, sl], pyr,
                                                gtt[:, 100 + t:101 + t],
                                                None, Alu.min)
                        nc.vector.tensor_scalar(T3y[:, sl], pyl,
                                                gtt[:, 150 + t:151 + t],
                                                None, Alu.max)
                OX4 = tmp.tile([NPART, 4 * F], f16, tag="OX4", bufs=2)
                nc.vector.tensor_tensor(OX4[:], T02[:], T13[:], Alu.subtract)
                OXR4 = tmp.tile([NPART, 4 * F], f16, tag="OXR4", bufs=2)
                nc.scalar.activation(OXR4[:], OX4[:], Act.Relu)
                OY4 = tmp.tile([NPART, 4 * F], f16, tag="OY4", bufs=2)
                nc.vector.tensor_tensor(OY4[:], T2y[:], T3y[:], Alu.subtract)
                PROD4 = tmp.tile([NPART, 4 * F], f16, tag="PROD4", bufs=2)
                nc.vector.tensor_tensor(PROD4[:], OXR4[:], OY4[:], Alu.mult)
                if po == 0:
                    U4 = accs[0]
                else:
                    U4 = tmp.tile([NPART, 4 * F], f16, tag="U4", bufs=2)
                for hi, t in enumerate(ts4):
                    sl = slice(hi * F, (hi + 1) * F)
                    negg = gtt[:, 300 + t:301 + t]
                    if hi % 2 == 0:
                        nc.scalar.activation(U4[:, sl], PROD4[:, sl],
                                             Act.Identity, bias=negg)
                    else:
                        nc.vector.tensor_scalar(U4[:, sl], PROD4[:, sl],
                                                negg, None, Alu.add)
                if po > 0:
                    # ping-pong: out never aliases an input (in-place max
                    # measured ~40% slower than a pure tensor_tensor)
                    cur, nxt = accs[(po - 1) % 2], accs[po % 2]
                    nc.vector.tensor_tensor(nxt[:], cur[:], U4[:], Alu.max)
            ACCF = accs[(MAXT // 4 - 1) % 2]
            # final fold: remaining 2 GTs (48, 49) on [F] tiles + acc merge
            AC2 = PC  # PC is dead after PR/PL; reuse its [2F] slot
            nc.vector.tensor_tensor(AC2[:], ACCF[:, 0:2 * F],
                                    ACCF[:, 2 * F:4 * F], Alu.max)
            for hi, t in enumerate((48, 49)):
                sl = slice(hi * F, (hi + 1) * F)
                t0 = tmp.tile([NPART, F], f16, tag="T02", bufs=2)
                nc.vector.tensor_scalar(t0[:], pxr, gtt[:, t:t + 1],
                                        gtt[:, 50 + t:51 + t],
                                        Alu.min, Alu.subtract)
                t1 = tmp.tile([NPART, F], f16, tag="T13", bufs=2)
                nc.scalar.activation(t1[:], pxl, Act.Relu,
                                     bias=gtt[:, 200 + t:201 + t])
                ox = tmp.tile([NPART, F], f16, tag="OX4", bufs=2)
                nc.vector.tensor_tensor(ox[:], t0[:], t1[:], Alu.subtract)
                oxr = tmp.tile([NPART, F], f16, tag="OXR4", bufs=2)
                nc.scalar.activation(oxr[:], ox[:], Act.Relu)
                t2 = tmp.tile([NPART, F], f16, tag="T2y", bufs=2)
                nc.vector.tensor_scalar(t2[:], pyr, gtt[:, 100 + t:101 + t],
                                        None, Alu.min)
                t3 = tmp.tile([NPART, F], f16, tag="T3y", bufs=2)
                nc.vector.tensor_scalar(t3[:], pyl, gtt[:, 150 + t:151 + t],
                                        None, Alu.max)
                oy = tmp.tile([NPART, F], f16, tag="OY4", bufs=2)
                nc.vector.tensor_tensor(oy[:], t2[:], t3[:], Alu.subtract)
                prod = tmp.tile([NPART, F], f16, tag="PROD4", bufs=2)
                nc.vector.tensor_tensor(prod[:], oxr[:], oy[:], Alu.mult)
                nc.vector.scalar_tensor_tensor(
                    AC2[:, sl], prod[:], gtt[:, 300 + t:301 + t],
                    AC2[:, sl], Alu.add, Alu.max)
            nc.vector.tensor_tensor(acc[:], AC2[:, 0:F], AC2[:, F:2 * F],
                                    Alu.max)

            # ---------- noobj sum ----------
            ind = per.tile([NPART, F], f16)   # 1.0 where max_iou <= 0.6
            nc.vector.tensor_tensor(ind[:], acc[:], thr[:], Alu.is_le)
            nc.vector.tensor_tensor(scrap[:], cf2m[:], ind[:], Alu.mult)
            nc.vector.tensor_reduce(rhs8[:, 0:1], scrap[:], X, Alu.add)

            # ---------- final assembly ----------
            ps8 = ps.tile([1, 8], f32)
            nc.tensor.matmul(ps8[:], ones[:], rhs8[:], start=True, stop=True)
            out_t = per.tile([1, 16], f32)
            nc.gpsimd.memset(out_t[:], 0.0)
            nc.scalar.activation(out_t[:, 1:2], ps8[:, 0:1], Act.Copy,
                                 scale=0.5 * NOOBJ_SCALE)
            nc.scalar.activation(out_t[:, 2:9], ps8[:, 1:8], Act.Copy)
            nc.vector.tensor_reduce(out_t[:, 0:1], out_t[:, 1:10], X, Alu.add)
            nc.gpsimd.dma_start(out=out_d[:, :], in_=out_t[:])
    nc.finalize()
    return nc


# ---------------- host-side preparation ----------------

def _iou_np(b1, b2):
    """center-format IoU, matches reference._iou_cc; broadcastable [...,4]"""
    mx = np.minimum(b1[..., 0] - 0.5 * b1[..., 2], b2[..., 0] - 0.5 * b2[..., 2])
    Mx = np.maximum(b1[..., 0] + 0.5 * b1[..., 2], b2[..., 0] + 0.5 * b2[..., 2])
    my = np.minimum(b1[..., 1] - 0.5 * b1[..., 3], b2[..., 1] - 0.5 * b2[..., 3])
    My = np.maximum(b1[..., 1] + 0.5 * b1[..., 3], b2[..., 1] + 0.5 * b2[..., 3])
    cw = b1[..., 2] + b2[..., 2] - (Mx - mx)
    ch = b1[..., 3] + b2[..., 3] - (My - my)
    inter = np.where((cw <= 0) | (ch <= 0), 0.0, cw * ch)
    union = b1[..., 2] * b1[..., 3] + b2[..., 2] * b2[..., 3] - inter
    return inter / union


def _pad_cells(arr_img):
    """[BPC, CPI] -> [128, F]: pad each image's cells to PPI*F, chunk."""
    out = np.zeros((BPC, PPI * F), arr_img.dtype)
    out[:, :CPI] = arr_img
    return out.reshape(BPC * PPI, F)


# static per-cell constants (same for every core/batch)
_CONSTS_CACHE = {}


def _static_consts():
    if "c" in _CONSTS_CACHE:
        return _CONSTS_CACHE["c"]
    f16 = np.float16
    hwg = np.arange(HW)
    col1 = (hwg % NW).astype(f16)
    row1 = (hwg // NW).astype(f16)
    colc = np.tile(col1, NA)                      # [CPI]
    rowc = np.tile(row1, NA)
    awcc = np.repeat((AW / 2.0).astype(f16), HW)  # [CPI]
    ahcc = np.repeat((AH / 2.0).astype(f16), HW)
    cc = {"col": colc, "row": rowc, "aw": awcc, "ah": ahcc}
    _CONSTS_CACHE["c"] = cc
    return cc


def _prep_core(out_np, tgt_np):
    """Build all device input tensors for one core (4 images)."""
    f32, f16 = np.float32, np.float16
    tgt = tgt_np.reshape(BPC, MAXT, 5).astype(f32)
    gx = tgt[:, :, 1] * NW
    gy = tgt[:, :, 2] * NH
    gw = tgt[:, :, 3] * NW
    gh = tgt[:, :, 4] * NH
    gcls = tgt[:, :, 0].astype(np.int32)
    valid = np.cumprod((tgt[:, :, 1] != 0).astype(np.int32), axis=1).astype(bool)

    # best anchor per gt by shape-only IoU (same math as reference)
    gt_shape = np.stack([np.zeros_like(gw), np.zeros_like(gw), gw, gh], -1)
    anc_box = np.stack([np.zeros(NA, f32), np.zeros(NA, f32),
                        AW.astype(f32), AH.astype(f32)], -1)
    a_ious = _iou_np(gt_shape[:, :, None, :], anc_box[None, None, :, :])
    best_n = np.argmax(a_ious, axis=-1)

    gi = gx.astype(np.int32)
    gj = gy.astype(np.int32)

    # gtt [128, 384]: cols [0:50) gxr | [50:100) gxl | [100:150) gyr
    #   [150:200) gyl | [200:250) -gxl | [250:300) -gyl | [300:350) -0.375*ga
    gxr_v = np.where(valid, gx + 0.5 * gw, -1.0e4)
    gxl_v = np.where(valid, gx - 0.5 * gw, 0.0)
    gyr_v = np.where(valid, gy + 0.5 * gh, -1.0e4)
    gyl_v = np.where(valid, gy - 0.5 * gh, 0.0)
    gtt_img = np.zeros((BPC, 384), f32)
    gtt_img[:, 0:MAXT] = gxr_v
    gtt_img[:, 50:50 + MAXT] = gxl_v
    gtt_img[:, 100:100 + MAXT] = gyr_v
    gtt_img[:, 150:150 + MAXT] = gyl_v
    gtt_img[:, 200:200 + MAXT] = -gxl_v
    gtt_img[:, 250:250 + MAXT] = -gyl_v
    gtt_img[:, 300:300 + MAXT] = np.where(valid, -0.375 * gw * gh, 0.0)
    gtt = np.repeat(gtt_img, PPI, axis=0)         # [128, 384]

    # scatter cells: last write wins per (b, best_n, gj, gi)
    cells = {}
    for b in range(BPC):
        for t in range(MAXT):
            if not valid[b, t]:
                continue
            key = (b, int(best_n[b, t]), int(gj[b, t]), int(gi[b, t]))
            cells[key] = t
    cell_list = list(cells.items())
    ncell = len(cell_list)
    assert ncell <= NCELL_CAP

    # per-cell constant tile [128, 5F] f16: [gcol | grow | aw/2 | ah/2 | mask]
    cc = _static_consts()
    constv = np.zeros((NPART, 5 * F), f16)
    constv[:, 0:F] = _pad_cells(np.broadcast_to(cc["col"], (BPC, CPI)))
    constv[:, F:2 * F] = _pad_cells(np.broadcast_to(cc["row"], (BPC, CPI)))
    aw_t = _pad_cells(np.broadcast_to(cc["aw"], (BPC, CPI)))
    ah_t = _pad_cells(np.broadcast_to(cc["ah"], (BPC, CPI)))
    aw_t[aw_t == 0] = 1.0   # pad cells: avoid 0 sizes
    ah_t[ah_t == 0] = 1.0
    constv[:, 2 * F:3 * F] = aw_t
    constv[:, 3 * F:4 * F] = ah_t
    mask_img = np.zeros((BPC, CPI), f16)
    mask_img[:, :] = 1.0
    for (b, a, j, i), _t in cell_list:
        mask_img[b, a * HW + j * NW + i] = 0.0
    constv[:, 4 * F:5 * F] = _pad_cells(mask_img)  # pads are already 0

    # channel tile [128, 5F] f32 from output: [x | y | conf | w | h]
    out_r = out_np.reshape(BPC, NA, 85, HW)
    chv = np.zeros((NPART, 5 * F), f32)
    for k, c in enumerate((0, 1, 4, 2, 3)):
        chv[:, k * F:(k + 1) * F] = _pad_cells(out_r[:, :, c, :].reshape(BPC, CPI))

    # gathered channels + per-cell aux for the small stage
    gathv = np.zeros((NCELL_CAP, 85), f32)
    auxcv = np.zeros((NCELL_CAP, 16), f32)
    onehv = np.zeros((NCELL_CAP, NCLS), f32)
    auxcv[:, 8] = 1.0  # pad rows: garea=1 avoids 0-union
    for s, ((b, a, j, i), t) in enumerate(cell_list):
        hw = j * NW + i
        ch = out_r[b, a, :, hw]
        gathv[s, 0] = ch[0]
        gathv[s, 1] = ch[1]
        gathv[s, 2] = ch[4]
        gathv[s, 3] = ch[2]
        gathv[s, 4] = ch[3]
        gathv[s, 5:] = ch[5:]
        bn = a
        auxcv[s, 0] = i
        auxcv[s, 1] = j
        auxcv[s, 2] = math.log(AW[bn] / 2.0)
        auxcv[s, 3] = math.log(AH[bn] / 2.0)
        auxcv[s, 4] = gx[b, t] - 0.5 * gw[b, t]
        auxcv[s, 5] = gx[b, t] + 0.5 * gw[b, t]
        auxcv[s, 6] = gy[b, t] - 0.5 * gh[b, t]
        auxcv[s, 7] = gy[b, t] + 0.5 * gh[b, t]
        auxcv[s, 8] = gw[b, t] * gh[b, t]
        auxcv[s, 9] = gx[b, t] - float(gi[b, t])
        auxcv[s, 10] = gy[b, t] - float(gj[b, t])
        auxcv[s, 11] = math.log(gw[b, t] / AW[bn])
        auxcv[s, 12] = math.log(gh[b, t] / AH[bn])
        auxcv[s, 13] = 1.0
        onehv[s, gcls[b, t]] = 1.0

    return {
        "chans": chv, "consts": constv, "gtt": gtt,
        "gath": gathv, "auxc": auxcv, "oneh": onehv,
    }


def kernel(output, target):
    from concourse.bass_utils import run_bass_kernel_spmd

    output = np.asarray(output, dtype=np.float32)
    target = np.asarray(target, dtype=np.float32)

    if "nc" not in _PROG_CACHE:
        _PROG_CACHE["nc"] = _build_program()
    nc = _PROG_CACHE["nc"]

    in_maps = []
    for core in range(NCORES):
        sl = slice(core * BPC, (core + 1) * BPC)
        in_maps.append(_prep_core(output[sl], target[sl]))

    res = run_bass_kernel_spmd(nc, in_maps, list(range(NCORES)))
    total = np.float32(0.0)
    for core in range(NCORES):
        total += np.float32(res.results[core]["out"][0, 0])
    return np.float32(total)


# revision 50
# speedup vs baseline: 1.0618x; 1.0618x over previous
"""RegionLoss (YOLOv2) Trainium2 kernel — 8-core batch-parallel SPMD, v2.

Contract: kernel(**inputs) takes FULL inputs (output [32,425,76,76] f32,
target [32,250] f32) and returns the FULL scalar loss, matching
reference.region_loss. Batch is sharded 4 images/core across 8 NeuronCores;
each core computes its partial loss on device; host sums the 8 partials.

Layout/engine strategy (derived from perfetto traces across 9 variants):
 - GpSimd(Pool) shares SBUF read/write ports with DVE: any concurrent Pool
   op drops DVE throughput up to 5x. Pool only issues DMAs (no ALU work).
 - DVE TensorScalar ops reach ~0.3ns/elem on packed fp16 (4x mode, incl.
   per-partition-scalar fetch); TensorTensor is capped at 2x_1p
   (~0.5ns/elem, ~0.42 at 2F width); ACT costs ~1.1ns/elem regardless.
   Per-op costs inflate ~16% when both DVE and ACT run >90% busy, so the
   engine split matters more than naive balance.
 - The 50-GT loop processes GTs in QUADS (48 GTs; 2 tail GTs at F): the
   per-GT tensor_scalar/ACT ops (which need per-partition scalars) write
   quarters of [128, 4F] tiles; the scalar-free tensor_tensor ops (ox, oy,
   prod, acc-max) and the relu run once per quad at 4F width.
   Per GT t (x side; y symmetric, no relu):
     t0  = (pxr min Gxr_t) - Gxl_t     (DVE TS — both scalar slots fused)
     t1  = relu(pxl - Gxl_t)           (ACT Relu, bias = -Gxl_t)
     ox  = t0 - t1                     (= min(pxr,Gxr) - max(pxl,Gxl))
     oxr = relu(ox)                    (ACT, quad 4F)
     prod= oxr * oy                    (DVE TT, quad 4F)
     u   = prod - 0.375*garea_t        (half ACT Identity+bias, half DVE)
     acc = max(acc, u)                 (DVE TT, quad 4F accumulator)
   suppression test: acc > 1.5*phw*phh  <=>  max-IoU > 0.6 (division-free,
   exact: iou>0.6 <=> inter > 0.375*(parea+garea)).
 - All 128 partitions active: partitions are image-major (32/image), each
   partition holds F=904 consecutive cells of the image's (anchor, hw)
   flattened plane; anchor constants enter as per-cell fp16 tiles.
 - DMAs are split per-channel and queued loop-critical-first (x, y, grid).
 - Activation tables: the 3 Sigmoids run back-to-back first; everything
   else (Exp, Ln, Relu, Identity, Square) shares natural_log_exp's table.
 - The matched-cell "small stage" is issued before the loop so the
   scheduler drops its ops into early engine bubbles.

Everything derived from `target` (gt boxes, best anchors, scatter cells,
masks) is host-precomputed metadata passed as small input tensors; the
<=50 matched cells per image are host-gathered (layout only) and their
coord/conf/class terms computed on device in f32 (small stage).
"""

import math
import numpy as np

# ---- problem constants (hardcoded per contract) ----
NB, NH, NW = 32, 76, 76
NA, NCLS = 5, 80
MAXT = 50
ANCHORS = np.array([1.3221, 1.73145, 3.19275, 4.00944, 5.05587, 8.09892,
                    9.47112, 4.84053, 11.2364, 10.0071], dtype=np.float32)
AW = ANCHORS.reshape(NA, 2)[:, 0]
AH = ANCHORS.reshape(NA, 2)[:, 1]
COORD_SCALE, NOOBJ_SCALE, OBJ_SCALE, CLASS_SCALE = 1.0, 1.0, 5.0, 1.0
THRESH = 0.6

NCORES = 8
BPC = NB // NCORES          # 4 images per core
HW = NH * NW                # 5776
CPI = NA * HW               # 28880 cells per image
PPI = 32                    # partitions per image
F = 904                     # cells per partition; 32*904 = 28928 >= 28880
NPART = 128
NCELL_CAP = 256             # small-stage cell capacity (2 x 128)

_PROG_CACHE = {}


def _build_program():
    import concourse.bacc as bacc
    import concourse.mybir as mybir
    from concourse.tile import TileContext

    f32 = mybir.dt.float32
    f16 = mybir.dt.float16
    Alu = mybir.AluOpType
    Act = mybir.ActivationFunctionType
    X = mybir.AxisListType.X

    nc = bacc.Bacc()

    # ---- I/O ----
    # chans columns: [x | y | conf | w | h] * F
    # consts columns: [gcol | grow | aw/2 | ah/2 | mask] * F
    chans = nc.declare_dram_parameter("chans", [NPART, 5 * F], f32, isOutput=False)
    consts = nc.declare_dram_parameter("consts", [NPART, 5 * F], f16, isOutput=False)
    gtt_d = nc.declare_dram_parameter("gtt", [NPART, 384], f32, isOutput=False)
    gath = nc.declare_dram_parameter("gath", [NCELL_CAP, 85], f32, isOutput=False)
    auxc = nc.declare_dram_parameter("auxc", [NCELL_CAP, 16], f32, isOutput=False)
    oneh = nc.declare_dram_parameter("oneh", [NCELL_CAP, NCLS], f32, isOutput=False)
    out_d = nc.declare_dram_parameter("out", [1, 16], f32, isOutput=True)

    with TileContext(nc) as tc:
        with tc.tile_pool(name="per", bufs=1) as per, \
             tc.tile_pool(name="tmp", bufs=4) as tmp, \
             tc.tile_pool(name="ps", bufs=1, space="PSUM") as ps:

            # ---------- input DMAs (split across queues for overlap) ----------
            cht = per.tile([NPART, 5 * F], f32)
            cst = per.tile([NPART, 5 * F], f16)
            gtt = per.tile([NPART, 384], f32)

            def chd(k):
                nc.gpsimd.dma_start(out=cht[:, k * F:(k + 1) * F],
                                    in_=chans[:, k * F:(k + 1) * F])

            def csd(k):
                nc.gpsimd.dma_start(out=cst[:, k * F:(k + 1) * F],
                                    in_=consts[:, k * F:(k + 1) * F])

            # priority order: loop-critical tensors first
            chd(0); chd(1); csd(0); csd(1)          # x, y, gcol, grow
            chd(3); chd(4); csd(2); csd(3)          # w, h, aw/2, ah/2
            nc.gpsimd.dma_start(out=gtt[:], in_=gtt_d[:, :])
            chd(2); csd(4)                          # conf, mask (end-only)
            g_ts, a_ts, o_ts = [], [], []
            for half in range(2):
                rows = slice(half * 128, (half + 1) * 128)
                g_t = per.tile([128, 85], f32, name=f"g_{half}")
                nc.gpsimd.dma_start(out=g_t[:], in_=gath[rows, :])
                a_t = per.tile([128, 16], f32, name=f"a_{half}")
                nc.gpsimd.dma_start(out=a_t[:], in_=auxc[rows, :])
                o_t = per.tile([128, NCLS], f32, name=f"o_{half}")
                nc.gpsimd.dma_start(out=o_t[:], in_=oneh[rows, :])
                g_ts.append(g_t); a_ts.append(a_t); o_ts.append(o_t)

            # ---------- hoist: batched ACT ops (xy first — loop-critical) ----------
            sxyc = per.tile([NPART, 3 * F], f16)   # sigmoid(x|y|conf)
            nc.scalar.activation(sxyc[:, 0:F], cht[:, 0:F], Act.Sigmoid)
            nc.scalar.activation(sxyc[:, F:2 * F], cht[:, F:2 * F],
                                 Act.Sigmoid)
            nc.scalar.activation(sxyc[:, 2 * F:3 * F], cht[:, 2 * F:3 * F],
                                 Act.Sigmoid)
            ewh = per.tile([NPART, 2 * F], f16)    # exp(w|h)
            nc.scalar.activation(ewh[:], cht[:, 3 * F:5 * F], Act.Exp)

            # wide DVE assembly: [x|y]-paired tiles
            PW = per.tile([NPART, 2 * F], f16)     # [pw/2 | ph/2]
            nc.vector.tensor_tensor(PW[:], ewh[:], cst[:, 2 * F:4 * F],
                                    Alu.mult)
            PC = per.tile([NPART, 2 * F], f16)     # [pxc | pyc]
            nc.vector.tensor_tensor(PC[:], sxyc[:, 0:2 * F], cst[:, 0:2 * F],
                                    Alu.add)
            PR = per.tile([NPART, 2 * F], f16)     # [pxr | pyr]
            nc.vector.tensor_tensor(PR[:], PC[:], PW[:], Alu.add)
            PL = per.tile([NPART, 2 * F], f16)     # [pxl | pyl]
            nc.vector.tensor_tensor(PL[:], PC[:], PW[:], Alu.subtract)
            pxr, pyr = PR[:, 0:F], PR[:, F:2 * F]
            pxl, pyl = PL[:, 0:F], PL[:, F:2 * F]
            # cf2m = (sigmoid(conf)*mask)^2 = conf^2 * mask  (mask is 0/1)
            smk = per.tile([NPART, F], f16)
            nc.vector.tensor_tensor(smk[:], sxyc[:, 2 * F:3 * F],
                                    cst[:, 4 * F:5 * F], Alu.mult)
            cf2m = per.tile([NPART, F], f16)
            nc.scalar.activation(cf2m[:], smk[:], Act.Square)
            pp = per.tile([NPART, F], f16)
            nc.vector.tensor_tensor(pp[:], PW[:, 0:F], PW[:, F:2 * F],
                                    Alu.mult)
            thr = per.tile([NPART, F], f16)   # 1.5*phw*phh
            nc.vector.tensor_scalar(thr[:], pp[:], 1.5, None, Alu.mult)

            scrap = sxyc[:, 0:F]  # sxyc is dead after PC/smk; reuse
            rhs8 = per.tile([NPART, 8], f32)  # all partial sums, one matmul
            nc.gpsimd.memset(rhs8[:], 0.0)
            ones = per.tile([NPART, 1], f32)
            nc.gpsimd.memset(ones[:], 1.0)

            # ---------- small stage: matched cells (DVE + ACT, f32) ----------
            for half in range(2):
                g_t, a_t, o_t = g_ts[half], a_ts[half], o_ts[half]
                # gath cols: 0 x | 1 y | 2 conf | 3 w | 4 h | 5:85 cls
                # auxc cols: 0 gi | 1 gj | 2 lnawh | 3 lnahh | 4 gxl | 5 gxr
                #            6 gyl | 7 gyr | 8 garea | 9 tx | 10 ty | 11 tw
                #            12 th | 13 valid
                # sigmoid via exp table: sig = 1/(1+exp(-v))
                nexp3 = per.tile([128, 3], f32, name=f"nexp3_{half}")
                nc.scalar.activation(nexp3[:], g_t[:, 0:3], Act.Exp,
                                     scale=-1.0)
                d3 = per.tile([128, 3], f32, name=f"d3_{half}")
                nc.vector.tensor_scalar(d3[:], nexp3[:], 1.0, None, Alu.add)
                sig3 = per.tile([128, 3], f32, name=f"sig3_{half}")
                nc.vector.reciprocal(sig3[:], d3[:])
                sphw = per.tile([128, 1], f32, name=f"sphw_{half}")
                nc.scalar.activation(sphw[:], g_t[:, 3:4], Act.Exp,
                                     bias=a_t[:, 2:3])
                sphh = per.tile([128, 1], f32, name=f"sphh_{half}")
                nc.scalar.activation(sphh[:], g_t[:, 4:5], Act.Exp,
                                     bias=a_t[:, 3:4])
                px = per.tile([128, 1], f32, name=f"px_{half}")
                nc.vector.tensor_scalar(px[:], sig3[:, 0:1], a_t[:, 0:1],
                                        None, Alu.add)
                py = per.tile([128, 1], f32, name=f"py_{half}")
                nc.vector.tensor_scalar(py[:], sig3[:, 1:2], a_t[:, 1:2],
                                        None, Alu.add)
                # overlap x
                spxr = per.tile([128, 1], f32, name=f"spxr_{half}")
                nc.vector.tensor_tensor(spxr, px[:], sphw[:], Alu.add)
                spxl = per.tile([128, 1], f32, name=f"spxl_{half}")
                nc.vector.tensor_tensor(spxl, px[:], sphw[:], Alu.subtract)
                st0 = per.tile([128, 1], f32, name=f"st0_{half}")
                nc.vector.tensor_scalar(st0[:], spxr, a_t[:, 5:6],
                                        None, Alu.min)
                st1 = per.tile([128, 1], f32, name=f"st1_{half}")
                nc.vector.tensor_scalar(st1[:], spxl, a_t[:, 4:5],
                                        None, Alu.max)
                sox = per.tile([128, 1], f32, name=f"sox_{half}")
                nc.vector.tensor_tensor(sox[:], st0[:], st1[:], Alu.subtract)
                soxr = per.tile([128, 1], f32, name=f"soxr_{half}")
                nc.vector.tensor_scalar(soxr[:], sox[:], 0.0, None, Alu.max)
                # overlap y
                pyr2 = per.tile([128, 1], f32, name=f"pyr2_{half}")
                nc.vector.tensor_tensor(pyr2[:], py[:], sphh[:], Alu.add)
                pyl2 = per.tile([128, 1], f32, name=f"pyl2_{half}")
                nc.vector.tensor_tensor(pyl2[:], py[:], sphh[:], Alu.subtract)
                st2 = per.tile([128, 1], f32, name=f"st2_{half}")
                nc.vector.tensor_scalar(st2[:], pyr2[:], a_t[:, 7:8],
                                        None, Alu.min)
                st3 = per.tile([128, 1], f32, name=f"st3_{half}")
                nc.vector.tensor_scalar(st3[:], pyl2[:], a_t[:, 6:7],
                                        None, Alu.max)
                soy = per.tile([128, 1], f32, name=f"soy_{half}")
                nc.vector.tensor_tensor(soy[:], st2[:], st3[:], Alu.subtract)
                soyr = per.tile([128, 1], f32, name=f"soyr_{half}")
                nc.vector.tensor_scalar(soyr[:], soy[:], 0.0, None, Alu.max)

                inter = per.tile([128, 1], f32, name=f"inter_{half}")
                nc.vector.tensor_tensor(inter[:], soxr[:], soyr[:], Alu.mult)
                pa = per.tile([128, 1], f32, name=f"pa_{half}")
                nc.vector.tensor_tensor(pa[:], sphw[:], sphh[:], Alu.mult)
                un = per.tile([128, 1], f32, name=f"un_{half}")
                nc.vector.tensor_scalar(un[:], pa[:], 4.0, a_t[:, 8:9],
                                        Alu.mult, Alu.add)
                un2 = per.tile([128, 1], f32, name=f"un2_{half}")
                nc.vector.tensor_tensor(un2[:], un[:], inter[:], Alu.subtract)
                rec = per.tile([128, 1], f32, name=f"rec_{half}")
                nc.vector.reciprocal(rec[:], un2[:])
                tiou = per.tile([128, 1], f32, name=f"tiou_{half}")
                nc.vector.tensor_tensor(tiou[:], inter[:], rec[:], Alu.mult)

                ctb = per.tile([128, 3], f32, name=f"ctb_{half}")
                # coord: 0.5*((sx-tx)^2+(sy-ty)^2+(w-tw)^2+(h-th)^2)
                scr = per.tile([128, 4], f32, name=f"scr_{half}")
                nc.vector.tensor_tensor(scr[:, 0:2], sig3[:, 0:2],
                                        a_t[:, 9:11], Alu.subtract)
                nc.vector.tensor_tensor(scr[:, 2:4], g_t[:, 3:5],
                                        a_t[:, 11:13], Alu.subtract)
                sq4 = per.tile([128, 4], f32, name=f"sq4_{half}")
                nc.scalar.activation(sq4[:], scr[:], Act.Square,
                                     scale=math.sqrt(0.5 * COORD_SCALE))
                nc.vector.tensor_reduce(ctb[:, 0:1], sq4[:], X, Alu.add)
                # obj conf: 2.5*(sconf - tiou)^2
                dcf = per.tile([128, 1], f32, name=f"dcf_{half}")
                nc.vector.tensor_tensor(dcf[:], sig3[:, 2:3], tiou[:],
                                        Alu.subtract)
                nc.scalar.activation(ctb[:, 1:2], dcf[:], Act.Square,
                                     scale=math.sqrt(0.5 * OBJ_SCALE))
                # class CE: logsumexp(cls) - <cls, onehot>
                mx = per.tile([128, 1], f32, name=f"mx_{half}")
                nc.vector.tensor_reduce(mx[:], g_t[:, 5:85], X, Alu.max)
                nmx = per.tile([128, 1], f32, name=f"nmx_{half}")
                nc.vector.tensor_scalar(nmx[:], mx[:], -1.0, None, Alu.mult)
                esc = per.tile([128, NCLS], f32, name=f"esc_{half}")
                sume = per.tile([128, 1], f32, name=f"sume_{half}")
                nc.scalar.activation(esc[:], g_t[:, 5:85], Act.Exp,
                                     bias=nmx[:])
                nc.vector.tensor_reduce(sume[:], esc[:], X, Alu.add)
                lns = per.tile([128, 1], f32, name=f"lns_{half}")
                nc.scalar.activation(lns[:], sume[:], Act.Ln)
                lse = per.tile([128, 1], f32, name=f"lse_{half}")
                nc.vector.tensor_tensor(lse[:], lns[:], mx[:], Alu.add)
                tgl = per.tile([128, NCLS], f32, name=f"tgl_{half}")
                tgv = per.tile([128, 1], f32, name=f"tgv_{half}")
                nc.vector.tensor_tensor(tgl[:], g_t[:, 5:85], o_t[:], Alu.mult)
                nc.vector.tensor_reduce(tgv[:], tgl[:], X, Alu.add)
                nc.vector.tensor_tensor(ctb[:, 2:3], lse[:], tgv[:],
                                        Alu.subtract)
                nc.vector.tensor_scalar(rhs8[:, 1 + 3 * half:4 + 3 * half],
                                        ctb[:], a_t[:, 13:14], None, Alu.mult)


            acc = smk  # smk is dead after cf2m; reuse its [F] slot

            # ---------- 50-gt inner loop (DVE + ACT only) ----------
            # gtt columns: [0:50) gxr | [50:100) gxl | [100:150) gyr
            #   [150:200) gyl | [200:250) -gxl | [250:300) -gyl
            #   [300:350) -0.375*gw*gh
            # GTs processed in QUADS: per-GT tensor_scalar/ACT ops (which
            # need per-partition scalars) write quarters of [128, 4F]
            # tiles; the scalar-free tensor_tensor ops (ox, oy, prod,
            # acc-max) and the relu run once per quad at 4F width.
            ACC4 = per.tile([NPART, 4 * F], f16)
            ACC4b = per.tile([NPART, 4 * F], f16)
            accs = [ACC4, ACC4b]
            for po in range(MAXT // 4):
                ts4 = (4 * po, 4 * po + 1, 4 * po + 2, 4 * po + 3)
                T02 = tmp.tile([NPART, 4 * F], f16, tag="T02", bufs=2)
                T13 = tmp.tile([NPART, 4 * F], f16, tag="T13", bufs=2)
                T2y = tmp.tile([NPART, 4 * F], f16, tag="T2y", bufs=2)
                T3y = tmp.tile([NPART, 4 * F], f16, tag="T3y", bufs=2)
                for hi, t in enumerate(ts4):
                    sl = slice(hi * F, (hi + 1) * F)
                    nc.vector.tensor_scalar(T02[:, sl], pxr, gtt[:, t:t + 1],
                                            gtt[:, 50 + t:51 + t],
                                            Alu.min, Alu.subtract)
                    nc.scalar.activation(T13[:, sl], pxl, Act.Relu,
                                         bias=gtt[:, 200 + t:201 + t])
                    if t % 7 < 4:
                        nc.vector.tensor_scalar(T2y[:, sl], pyr,
                                                gtt[:, 100 + t:101 + t],
                                                gtt[:, 150 + t:151 + t],
                                                Alu.min, Alu.subtract)
                        nc.scalar.activation(T3y[:, sl], pyl, Act.Relu,
                                             bias=gtt[:, 250 + t:251 + t])
                    else:
                        nc.vector.tensor_scalar(T2y[:, sl], pyr,
                                                gtt[:, 100 + t:101 + t],
                                                None, Alu.min)
                        nc.vector.tensor_scalar(T3y[:, sl], pyl,
                                                gtt[:, 150 + t:151 + t],
                                                None, Alu.max)
                OX4 = tmp.tile([NPART, 4 * F], f16, tag="OX4", bufs=2)
                nc.vector.tensor_tensor(OX4[:], T02[:], T13[:], Alu.subtract)
                OXR4 = tmp.tile([NPART, 4 * F], f16, tag="OXR4", bufs=2)
                nc.scalar.activation(OXR4[:], OX4[:], Act.Relu)
                OY4 = tmp.tile([NPART, 4 * F], f16, tag="OY4", bufs=2)
                nc.vector.tensor_tensor(OY4[:], T2y[:], T3y[:], Alu.subtract)
                PROD4 = tmp.tile([NPART, 4 * F], f16, tag="PROD4", bufs=2)
                nc.vector.tensor_tensor(PROD4[:], OXR4[:], OY4[:], Alu.mult)
                if po == 0:
                    U4 = accs[0]
                else:
                    U4 = tmp.tile([NPART, 4 * F], f16, tag="U4", bufs=2)
                for hi, t in enumerate(ts4):
                    sl = slice(hi * F, (hi + 1) * F)
                    negg = gtt[:, 300 + t:301 + t]
                    if hi % 2 == 0:
                        nc.scalar.activation(U4[:, sl], PROD4[:, sl],
                                             Act.Identity, bias=negg)
                    else:
                        nc.vector.tensor_scalar(U4[:, sl], PROD4[:, sl],
                                                negg, None, Alu.add)
                if po > 0:
                    # ping-pong: out never aliases an input (in-place max
                    # measured ~40% slower than a pure tensor_tensor)
                    cur, nxt = accs[(po - 1) % 2], accs[po % 2]
                    nc.vector.tensor_tensor(nxt[:], cur[:], U4[:], Alu.max)
            ACCF = accs[(MAXT // 4 - 1) % 2]
            # final fold: remaining 2 GTs (48, 49) on [F] tiles + acc merge
            AC2 = PC  # PC is dead after PR/PL; reuse its [2F] slot
            nc.vector.tensor_tensor(AC2[:], ACCF[:, 0:2 * F],
                                    ACCF[:, 2 * F:4 * F], Alu.max)
            for hi, t in enumerate((48, 49)):
                sl = slice(hi * F, (hi + 1) * F)
                t0 = tmp.tile([NPART, F], f16, tag="T02", bufs=2)
                nc.vector.tensor_scalar(t0[:], pxr, gtt[:, t:t + 1],
                                        gtt[:, 50 + t:51 + t],
                                        Alu.min, Alu.subtract)
                t1 = tmp.tile([NPART, F], f16, tag="T13", bufs=2)
                nc.scalar.activation(t1[:], pxl, Act.Relu,
                                     bias=gtt[:, 200 + t:201 + t])
                ox = tmp.tile([NPART, F], f16, tag="OX4", bufs=2)
                nc.vector.tensor_tensor(ox[:], t0[:], t1[:], Alu.subtract)
                oxr = tmp.tile([NPART, F], f16, tag="OXR4", bufs=2)
                nc.scalar.activation(oxr[:], ox[:], Act.Relu)
                t2 = tmp.tile([NPART, F], f16, tag="T2y", bufs=2)
                nc.vector.tensor_scalar(t2[:], pyr, gtt[:, 100 + t:101 + t],
                                        None, Alu.min)
                t3 = tmp.tile([NPART, F], f16, tag="T3y", bufs=2)
                nc.vector.tensor_scalar(t3[:], pyl, gtt[:, 150 + t:151 + t],
                                        None, Alu.max)
                oy = tmp.tile([NPART, F], f16, tag="OY4", bufs=2)
                nc.vector.tensor_tensor(oy[:], t2[:], t3[:], Alu.subtract)
                prod = tmp.tile([NPART, F], f16, tag="PROD4", bufs=2)
                nc.vector.tensor_tensor(prod[:], oxr[:], oy[:], Alu.mult)
                nc.vector.scalar_tensor_tensor(
                    AC2[:, sl], prod[:], gtt[:, 300 + t:301 + t],
                    AC2[:, sl], Alu.add, Alu.max)
            nc.vector.tensor_tensor(acc[:], AC2[:, 0:F], AC2[:, F:2 * F],
                                    Alu.max)

            # ---------- noobj sum ----------
            ind = per.tile([NPART, F], f16)   # 1.0 where max_iou <= 0.6
            nc.vector.tensor_tensor(ind[:], acc[:], thr[:], Alu.is_le)
            nc.vector.tensor_tensor(scrap[:], cf2m[:], ind[:], Alu.mult)
            nc.vector.tensor_reduce(rhs8[:, 0:1], scrap[:], X, Alu.add)

            # ---------- final assembly ----------
            ps8 = ps.tile([1, 8], f32)
            nc.tensor.matmul(ps8[:], ones[:], rhs8[:], start=True, stop=True)
            out_t = per.tile([1, 16], f32)
            nc.gpsimd.memset(out_t[:], 0.0)
            nc.scalar.activation(out_t[:, 1:2], ps8[:, 0:1], Act.Copy,
                                 scale=0.5 * NOOBJ_SCALE)
            nc.scalar.activation(out_t[:, 2:9], ps8[:, 1:8], Act.Copy)
            nc.vector.tensor_reduce(out_t[:, 0:1], out_t[:, 1:10], X, Alu.add)
            nc.gpsimd.dma_start(out=out_d[:, :], in_=out_t[:])
    nc.finalize()
    return nc


# ---------------- host-side preparation ----------------

def _iou_np(b1, b2):
    """center-format IoU, matches reference._iou_cc; broadcastable [...,4]"""
    mx = np.minimum(b1[..., 0] - 0.5 * b1[..., 2], b2[..., 0] - 0.5 * b2[..., 2])
    Mx = np.maximum(b1[..., 0] + 0.5 * b1[..., 2], b2[..., 0] + 0.5 * b2[..., 2])
    my = np.minimum(b1[..., 1] - 0.5 * b1[..., 3], b2[..., 1] - 0.5 * b2[..., 3])
    My = np.maximum(b1[..., 1] + 0.5 * b1[..., 3], b2[..., 1] + 0.5 * b2[..., 3])
    cw = b1[..., 2] + b2[..., 2] - (Mx - mx)
    ch = b1[..., 3] + b2[..., 3] - (My - my)
    inter = np.where((cw <= 0) | (ch <= 0), 0.0, cw * ch)
    union = b1[..., 2] * b1[..., 3] + b2[..., 2] * b2[..., 3] - inter
    return inter / union


def _pad_cells(arr_img):
    """[BPC, CPI] -> [128, F]: pad each image's cells to PPI*F, chunk."""
    out = np.zeros((BPC, PPI * F), arr_img.dtype)
    out[:, :CPI] = arr_img
    return out.reshape(BPC * PPI, F)


# static per-cell constants (same for every core/batch)
_CONSTS_CACHE = {}


def _static_consts():
    if "c" in _CONSTS_CACHE:
        return _CONSTS_CACHE["c"]
    f16 = np.float16
    hwg = np.arange(HW)
    col1 = (hwg % NW).astype(f16)
    row1 = (hwg // NW).astype(f16)
    colc = np.tile(col1, NA)                      # [CPI]
    rowc = np.tile(row1, NA)
    awcc = np.repeat((AW / 2.0).astype(f16), HW)  # [CPI]
    ahcc = np.repeat((AH / 2.0).astype(f16), HW)
    cc = {"col": colc, "row": rowc, "aw": awcc, "ah": ahcc}
    _CONSTS_CACHE["c"] = cc
    return cc


def _prep_core(out_np, tgt_np):
    """Build all device input tensors for one core (4 images)."""
    f32, f16 = np.float32, np.float16
    tgt = tgt_np.reshape(BPC, MAXT, 5).astype(f32)
    gx = tgt[:, :, 1] * NW
    gy = tgt[:, :, 2] * NH
    gw = tgt[:, :, 3] * NW
    gh = tgt[:, :, 4] * NH
    gcls = tgt[:, :, 0].astype(np.int32)
    valid = np.cumprod((tgt[:, :, 1] != 0).astype(np.int32), axis=1).astype(bool)

    # best anchor per gt by shape-only IoU (same math as reference)
    gt_shape = np.stack([np.zeros_like(gw), np.zeros_like(gw), gw, gh], -1)
    anc_box = np.stack([np.zeros(NA, f32), np.zeros(NA, f32),
                        AW.astype(f32), AH.astype(f32)], -1)
    a_ious = _iou_np(gt_shape[:, :, None, :], anc_box[None, None, :, :])
    best_n = np.argmax(a_ious, axis=-1)

    gi = gx.astype(np.int32)
    gj = gy.astype(np.int32)

    # gtt [128, 384]: cols [0:50) gxr | [50:100) gxl | [100:150) gyr
    #   [150:200) gyl | [200:250) -gxl | [250:300) -gyl | [300:350) -0.375*ga
    gxr_v = np.where(valid, gx + 0.5 * gw, -1.0e4)
    gxl_v = np.where(valid, gx - 0.5 * gw, 0.0)
    gyr_v = np.where(valid, gy + 0.5 * gh, -1.0e4)
    gyl_v = np.where(valid, gy - 0.5 * gh, 0.0)
    gtt_img = np.zeros((BPC, 384), f32)
    gtt_img[:, 0:MAXT] = gxr_v
    gtt_img[:, 50:50 + MAXT] = gxl_v
    gtt_img[:, 100:100 + MAXT] = gyr_v
    gtt_img[:, 150:150 + MAXT] = gyl_v
    gtt_img[:, 200:200 + MAXT] = -gxl_v
    gtt_img[:, 250:250 + MAXT] = -gyl_v
    gtt_img[:, 300:300 + MAXT] = np.where(valid, -0.375 * gw * gh, 0.0)
    gtt = np.repeat(gtt_img, PPI, axis=0)         # [128, 384]

    # scatter cells: last write wins per (b, best_n, gj, gi)
    cells = {}
    for b in range(BPC):
        for t in range(MAXT):
            if not valid[b, t]:
                continue
            key = (b, int(best_n[b, t]), int(gj[b, t]), int(gi[b, t]))
            cells[key] = t
    cell_list = list(cells.items())
    ncell = len(cell_list)
    assert ncell <= NCELL_CAP

    # per-cell constant tile [128, 5F] f16: [gcol | grow | aw/2 | ah/2 | mask]
    cc = _static_consts()
    constv = np.zeros((NPART, 5 * F), f16)
    constv[:, 0:F] = _pad_cells(np.broadcast_to(cc["col"], (BPC, CPI)))
    constv[:, F:2 * F] = _pad_cells(np.broadcast_to(cc["row"], (BPC, CPI)))
    aw_t = _pad_cells(np.broadcast_to(cc["aw"], (BPC, CPI)))
    ah_t = _pad_cells(np.broadcast_to(cc["ah"], (BPC, CPI)))
    aw_t[aw_t == 0] = 1.0   # pad cells: avoid 0 sizes
    ah_t[ah_t == 0] = 1.0
    constv[:, 2 * F:3 * F] = aw_t
    constv[:, 3 * F:4 * F] = ah_t
    mask_img = np.zeros((BPC, CPI), f16)
    mask_img[:, :] = 1.0
    for (b, a, j, i), _t in cell_list:
        mask_img[b, a * HW + j * NW + i] = 0.0
    constv[:, 4 * F:5 * F] = _pad_cells(mask_img)  # pads are already 0

    # channel tile [128, 5F] f32 from output: [x | y | conf | w | h]
    out_r = out_np.reshape(BPC, NA, 85, HW)
    chv = np.zeros((NPART, 5 * F), f32)
    for k, c in enumerate((0, 1, 4, 2, 3)):
        chv[:, k * F:(k + 1) * F] = _pad_cells(out_r[:, :, c, :].reshape(BPC, CPI))

    # gathered channels + per-cell aux for the small stage
    gathv = np.zeros((NCELL_CAP, 85), f32)
    auxcv = np.zeros((NCELL_CAP, 16), f32)
    onehv = np.zeros((NCELL_CAP, NCLS), f32)
    auxcv[:, 8] = 1.0  # pad rows: garea=1 avoids 0-union
    for s, ((b, a, j, i), t) in enumerate(cell_list):
        hw = j * NW + i
        ch = out_r[b, a, :, hw]
        gathv[s, 0] = ch[0]
        gathv[s, 1] = ch[1]
        gathv[s, 2] = ch[4]
        gathv[s, 3] = ch[2]
        gathv[s, 4] = ch[3]
        gathv[s, 5:] = ch[5:]
        bn = a
        auxcv[s, 0] = i
        auxcv[s, 1] = j
        auxcv[s, 2] = math.log(AW[bn] / 2.0)
        auxcv[s, 3] = math.log(AH[bn] / 2.0)
        auxcv[s, 4] = gx[b, t] - 0.5 * gw[b, t]
        auxcv[s, 5] = gx[b, t] + 0.5 * gw[b, t]
        auxcv[s, 6] = gy[b, t] - 0.5 * gh[b, t]
        auxcv[s, 7] = gy[b, t] + 0.5 * gh[b, t]
        auxcv[s, 8] = gw[b, t] * gh[b, t]
        auxcv[s, 9] = gx[b, t] - float(gi[b, t])
        auxcv[s, 10] = gy[b, t] - float(gj[b, t])
        auxcv[s, 11] = math.log(gw[b, t] / AW[bn])
        auxcv[s, 12] = math.log(gh[b, t] / AH[bn])
        auxcv[s, 13] = 1.0
        onehv[s, gcls[b, t]] = 1.0

    return {
        "chans": chv, "consts": constv, "gtt": gtt,
        "gath": gathv, "auxc": auxcv, "oneh": onehv,
    }


def kernel(output, target):
    from concourse.bass_utils import run_bass_kernel_spmd

    output = np.asarray(output, dtype=np.float32)
    target = np.asarray(target, dtype=np.float32)

    if "nc" not in _PROG_CACHE:
        _PROG_CACHE["nc"] = _build_program()
    nc = _PROG_CACHE["nc"]

    in_maps = []
    for core in range(NCORES):
        sl = slice(core * BPC, (core + 1) * BPC)
        in_maps.append(_prep_core(output[sl], target[sl]))

    res = run_bass_kernel_spmd(nc, in_maps, list(range(NCORES)))
    total = np.float32(0.0)
    for core in range(NCORES):
        total += np.float32(res.results[core]["out"][0, 0])
    return np.float32(total)
